# revision 7
# baseline (speedup 1.0000x reference)
"""Contrastive loss (NT-Xent style) Bass kernel for Trainium2, 8 NeuronCores.

Reference math (embeddings E: (8192, 512) f32):
    unit = E / ||E||_row            (eps clamp never fires for this data)
    sim  = unit @ unit.T / 0.05     (symmetric)
    sum_exp = rowsum(exp(sim)) + colsum(exp(sim)) - diag(exp(sim))
            = 2*rowsum(exp(sim)) - exp(diag)        [symmetry]
    loss = mean_i [ ln(sum_exp[2i]) - ln(sim_diag[2i+1]) ]

Sharding: core c receives the embedding matrix rolled by -c*1024 rows, so
its own 1024 rows come first and the program is SPMD-uniform. Each core:
  - computes row norms in fp32, rn' = sqrt(20/||e||^2) via exp(0.5*ln(.)),
  - rescales rows by rn' and casts to bf16 (u' = unit/sqrt(temperature)),
  - transposes u' into U'^T (512 x 8192 bf16) with PE (matmul by identity),
  - GEMM: sim' = U'_blk @ U'^T in bf16 (fp32 PSUM accumulate),
  - masks its diagonal entries (chunk position is core-independent thanks
    to the roll) to 0 before exp, so the bf16 diagonal never enters the
    row sums,
  - exp + row-sum via ScalarE activation accum_out,
  - outputs row sums and d = ||u'||^2 (fp32 analytic diagonal, ~= 20).
Host finishes the tiny tail: sum_exp = 2*(rowsum-1) + exp(d),
loss_i = ln(sum_exp[even]) - ln(d[odd]), mean over 4096 pairs.
No collectives, no GPSIMD.
"""

import sys
import types

sys.path.insert(0, "/opt/trn_rl_repo")


def _install_ntff_hook():
    """antenv in this container lacks axon_hooks; inject it so trace=True
    can capture NTFF profiles. Harmless when tracing is unused."""
    import antenv

    if hasattr(antenv, "axon_hooks"):
        return
    mod = types.ModuleType("antenv.axon_hooks")
    mod._hook = None

    def set_axon_ntff_profile_hook(h):
        mod._hook = h

    def get_axon_ntff_profile_hook():
        return mod._hook

    mod.set_axon_ntff_profile_hook = set_axon_ntff_profile_hook
    mod.get_axon_ntff_profile_hook = get_axon_ntff_profile_hook
    sys.modules["antenv.axon_hooks"] = mod
    antenv.axon_hooks = mod
    try:
        from trn_agent_boot.trn_boot import _ntff_profile_via_ctypes

        hook = _ntff_profile_via_ctypes("/opt/axon/libaxon_pjrt.so")
        if hook is not None:
            set_axon_ntff_profile_hook(hook)
    except Exception:
        pass


_install_ntff_hook()

import numpy as np  # noqa: E402
import ml_dtypes  # noqa: E402

import concourse.bass as bass  # noqa: E402
import concourse.mybir as mybir  # noqa: E402
import concourse.tile as tile  # noqa: E402
from concourse import bacc  # noqa: E402
from concourse.bass_utils import run_bass_kernel_spmd  # noqa: E402

F32 = mybir.dt.float32
BF16 = mybir.dt.bfloat16
AF = mybir.ActivationFunctionType
ALU = mybir.AluOpType

TWO_N = 8192
D = 512
N_CORES = 8
BLK = TWO_N // N_CORES          # 1024 rows per core
P = 128                         # partitions
NCH = 512                       # psum-bank chunk of the N dimension
N_CHUNKS = TWO_N // NCH         # 16
M_TILES = BLK // P              # 8
K_TILES = D // P                # 4
FULL_T = TWO_N // P             # 64 row tiles in the full matrix
BLK_T = BLK // P                # 8 row tiles in the own block
TEMP = 0.05
INV_T = 1.0 / TEMP              # 20.0

_CACHE = {}


def _build():
    """Build + compile the SPMD program once per process."""
    if "nc" in _CACHE:
        return _CACHE["nc"]

    nc = bacc.Bacc("TRN2", target_bir_lowering=False, debug=False,
                   num_devices=N_CORES)

    emb = nc.dram_tensor("emb", [TWO_N, D], F32, kind="ExternalInput").ap()
    eblk = nc.dram_tensor("eblk", [BLK, D], F32, kind="ExternalInput").ap()
    masks = nc.dram_tensor("masks", [P, 4 * NCH], F32,
                           kind="ExternalInput").ap()
    identin = nc.dram_tensor("identin", [P, P], BF16,
                             kind="ExternalInput").ap()
    rsout = nc.dram_tensor("rsout", [P, M_TILES], F32,
                           kind="ExternalOutput").ap()
    dout = nc.dram_tensor("dout", [P, M_TILES], F32,
                          kind="ExternalOutput").ap()

    emb_t = emb.rearrange("(t p) d -> t p d", p=P)    # 64 x (128, 512)
    eblk_t = eblk.rearrange("(t p) d -> t p d", p=P)  # 8 x (128, 512)

    with tile.TileContext(nc) as tc:
        with (
            tc.tile_pool(name="persist", bufs=1) as pp,
            tc.tile_pool(name="work", bufs=6) as wp,
            tc.tile_pool(name="small", bufs=4) as sp,
            tc.tile_pool(name="tps", bufs=1, space="PSUM") as tps,
            tc.tile_pool(name="mmps", bufs=3, space="PSUM") as mmps,
        ):
            ident = pp.tile([P, P], BF16, tag="ident")
            nc.sync.dma_start(ident[:], identin[:])
            maskt = pp.tile([P, 4 * NCH], F32, tag="maskt")
            nc.sync.dma_start(maskt[:], masks[:])

            # U'^T[k][f, j] = emb[j, 128k + f] * sqrt(20)/||e_j||  (bf16)
            UT = [pp.tile([P, TWO_N], BF16, tag=f"ut{k}", name=f"ut{k}")
                  for k in range(K_TILES)]
            UTB = [pp.tile([P, BLK], BF16, tag=f"utb{k}", name=f"utb{k}")
                   for k in range(K_TILES)]
            # d[:, m] = ||u'||^2 per own row (~= 20), fp32
            dtile = pp.tile([P, M_TILES], F32, tag="d")
            rstile = pp.tile([P, M_TILES], F32, tag="rst")

            sqscr = pp.tile([P, NCH], F32, tag="sqscr")   # unread scratch
            escr = pp.tile([P, NCH], F32, tag="escr")     # unread scratch

            def load_normalize(src_ap, own_m):
                """DMA one row tile, return bf16 normalized u' tile."""
                et = wp.tile([P, D], F32, tag="et")
                nc.sync.dma_start(et[:], src_ap)
                nsq = sp.tile([P, 1], F32, tag="nsq")
                nc.vector.tensor_tensor_reduce(
                    out=sqscr[:], in0=et[:], in1=et[:], scale=1.0, scalar=0.0,
                    op0=ALU.mult, op1=ALU.add, accum_out=nsq[:],
                )
                rcp = sp.tile([P, 1], F32, tag="rcp")
                nc.vector.reciprocal(rcp[:], nsq[:])
                # rn' = sqrt(20/nsq) = exp(0.5*ln(20*rcp)) — keeps the whole
                # kernel on one ACT table set (natural_log_exp_and_others)
                lg = sp.tile([P, 1], F32, tag="lg")
                nc.scalar.activation(lg[:], rcp[:], AF.Ln, scale=INV_T)
                rnp = sp.tile([P, 1], F32, tag="rnp")
                nc.scalar.activation(rnp[:], lg[:], AF.Exp, scale=0.5)
                if own_m is not None:
                    rsq = sp.tile([P, 1], F32, tag="rsq")
                    nc.vector.tensor_mul(rsq[:], rnp[:], rnp[:])
                    nc.vector.tensor_mul(
                        dtile[:, own_m:own_m + 1], rsq[:], nsq[:])
                ub = wp.tile([P, D], BF16, tag="ub")
                # u' = e * rn'  (cast to bf16), per-partition scalar on DVE
                nc.vector.tensor_scalar_mul(ub[:], et[:], rnp[:])
                return ub

            def transpose_group(ubs, dest, gcol):
                """PE-transpose 4 u' tiles into dest[k][:, gcol*512 ...].
                Regular matmul against the identity: out = ub_sliceT @ I."""
                for k in range(K_TILES):
                    ps = tps.tile([P, NCH], F32, tag=f"tp{k}",
                                  name=f"tp{k}_{gcol}")
                    for a in range(4):
                        nc.tensor.matmul(
                            ps[:, a * P:(a + 1) * P],
                            lhsT=ubs[a][:, k * P:(k + 1) * P],
                            rhs=ident[:], start=True, stop=True)
                    nc.vector.tensor_copy(
                        dest[k][:, gcol * NCH:(gcol + 1) * NCH], ps[:])

            # Own block first (feeds lhsT + diagonals), then full matrix.
            for g in range(BLK_T // 4):
                ubs = [load_normalize(eblk_t[g * 4 + a], g * 4 + a)
                       for a in range(4)]
                transpose_group(ubs, UTB, g)
            for g in range(FULL_T // 4):
                ubs = [load_normalize(emb_t[g * 4 + a], None)
                       for a in range(4)]
                transpose_group(ubs, UT, g)

            # Main GEMM + diagonal mask + exp + rowsum
            for m in range(M_TILES):
                racc = sp.tile([P, N_CHUNKS], F32, tag="racc")
                for n in range(N_CHUNKS):
                    ps = mmps.tile([P, NCH], F32, tag="mm")
                    for k in range(K_TILES):
                        nc.tensor.matmul(
                            ps[:],
                            lhsT=UTB[k][:, m * P:(m + 1) * P],
                            rhs=UT[k][:, n * NCH:(n + 1) * NCH],
                            start=(k == 0), stop=(k == K_TILES - 1),
                        )
                    if n == m // 4:
                        # own diagonal lives here (input is rolled):
                        # zero it so exp contributes exactly 1
                        p4 = m % 4
                        nc.vector.tensor_mul(
                            ps[:], ps[:], maskt[:, p4 * NCH:(p4 + 1) * NCH])
                    nc.scalar.activation(
                        escr[:], ps[:], AF.Exp, accum_out=racc[:, n:n + 1])
                nc.vector.reduce_sum(rstile[:, m:m + 1], racc[:],
                                     mybir.AxisListType.X)

            nc.sync.dma_start(rsout[:], rstile[:])
            nc.sync.dma_start(dout[:], dtile[:])

    nc.compile()
    _CACHE["nc"] = nc
    return nc


def _host_inputs(embeddings: np.ndarray):
    emb = np.ascontiguousarray(np.asarray(embeddings, dtype=np.float32))
    assert emb.shape == (TWO_N, D)
    masks = np.ones((P, 4 * NCH), dtype=np.float32)
    r = np.arange(P)
    for p4 in range(4):
        masks[r, p4 * NCH + p4 * P + r] = 0.0
    ident = np.eye(P, dtype=np.float32).astype(ml_dtypes.bfloat16)
    in_maps = []
    for c in range(N_CORES):
        emb_c = np.roll(emb, -c * BLK, axis=0)
        in_maps.append({
            "emb": emb_c,
            "eblk": np.ascontiguousarray(emb_c[:BLK]),
            "masks": masks,
            "identin": ident,
        })
    return in_maps


def run(embeddings: np.ndarray, trace: bool = False):
    nc = _build()
    in_maps = _host_inputs(embeddings)
    res = run_bass_kernel_spmd(nc, in_maps, list(range(N_CORES)), trace=trace)
    total = 0.0
    for c in range(N_CORES):
        rs = np.asarray(res.results[c]["rsout"], dtype=np.float64)  # (128, 8)
        d = np.asarray(res.results[c]["dout"], dtype=np.float64)    # (128, 8)
        # sum_exp = 2*(rowsum - 1) + exp(d); masked diagonal contributed 1
        se = 2.0 * (rs - 1.0) + np.exp(d)
        total += np.log(se[0::2, :]).sum() - np.log(d[1::2, :]).sum()
    loss = np.array(total / (TWO_N // 2), dtype=np.float32)
    return loss, res


def kernel(embeddings: np.ndarray) -> np.ndarray:
    loss, _ = run(embeddings, trace=False)
    return loss


# revision 8
# speedup vs baseline: 1.0284x; 1.0284x over previous
"""Contrastive loss Bass kernel, minimal-construct variant.

Device (per core, SPMD): sim' = U'_blk @ U'^T in bf16 (fp32 PSUM), mask own
diagonal, exp + row-sum on ScalarE, write row sums. Host: normalize/cast/
transpose prep (0.02% of FLOPs) and the final log/mean tail.
Inputs are rolled per core so the program is SPMD-uniform.
"""

import sys
import types

sys.path.insert(0, "/opt/trn_rl_repo")


def _install_ntff_hook():
    import antenv

    if hasattr(antenv, "axon_hooks"):
        return
    mod = types.ModuleType("antenv.axon_hooks")
    mod._hook = None

    def set_axon_ntff_profile_hook(h):
        mod._hook = h

    def get_axon_ntff_profile_hook():
        return mod._hook

    mod.set_axon_ntff_profile_hook = set_axon_ntff_profile_hook
    mod.get_axon_ntff_profile_hook = get_axon_ntff_profile_hook
    sys.modules["antenv.axon_hooks"] = mod
    antenv.axon_hooks = mod
    try:
        from trn_agent_boot.trn_boot import _ntff_profile_via_ctypes

        hook = _ntff_profile_via_ctypes("/opt/axon/libaxon_pjrt.so")
        if hook is not None:
            set_axon_ntff_profile_hook(hook)
    except Exception:
        pass


_install_ntff_hook()

import numpy as np  # noqa: E402
import ml_dtypes  # noqa: E402

import concourse.mybir as mybir  # noqa: E402
import concourse.tile as tile  # noqa: E402
from concourse import bacc  # noqa: E402
from concourse.bass_utils import run_bass_kernel_spmd  # noqa: E402

F32 = mybir.dt.float32
BF16 = mybir.dt.bfloat16
AF = mybir.ActivationFunctionType
ALU = mybir.AluOpType

TWO_N = 8192
D = 512
N_CORES = 8
BLK = TWO_N // N_CORES          # 1024
P = 128
NCH = 512
N_CHUNKS = TWO_N // NCH         # 16
M_TILES = BLK // P              # 8
K_TILES = D // P                # 4
TEMP = 0.05

_CACHE = {}


def _build():
    if "nc" in _CACHE:
        return _CACHE["nc"]

    nc = bacc.Bacc("TRN2", target_bir_lowering=False, debug=False,
                   num_devices=N_CORES)

    # U'^T in bf16, k-major: ut[k] is (128, 8192), row f = feature 128k+f,
    # col j = (rolled) row j of u' = unit/sqrt(T)
    ut_in = [nc.dram_tensor(f"ut{k}", [P, TWO_N], BF16,
                            kind="ExternalInput").ap() for k in range(K_TILES)]
    masks = nc.dram_tensor("masks", [P, 4 * NCH], F32,
                           kind="ExternalInput").ap()
    rsout = nc.dram_tensor("rsout", [P, M_TILES], F32,
                           kind="ExternalOutput").ap()

    with tile.TileContext(nc) as tc:
        with (
            tc.tile_pool(name="persist", bufs=1) as pp,
            tc.tile_pool(name="small", bufs=4) as sp,
            tc.tile_pool(name="mmps", bufs=4, space="PSUM") as mmps,
        ):
            UT = [pp.tile([P, TWO_N], BF16, tag=f"ut{k}", name=f"utt{k}")
                  for k in range(K_TILES)]
            for k in range(K_TILES):
                # 4 x 2MB loads, chunked for DMA pipelining
                for q in range(4):
                    nc.sync.dma_start(
                        UT[k][:, q * (TWO_N // 4):(q + 1) * (TWO_N // 4)],
                        ut_in[k][:, q * (TWO_N // 4):(q + 1) * (TWO_N // 4)])
            maskt = pp.tile([P, 4 * NCH], F32, tag="maskt")
            nc.sync.dma_start(maskt[:], masks[:])
            rstile = pp.tile([P, M_TILES], F32, tag="rst")
            mscr = pp.tile([P, NCH], F32, tag="mscr")
            escr = pp.tile([P, NCH], F32, tag="escr")

            for m in range(M_TILES):
                racc = sp.tile([P, N_CHUNKS], F32, tag="racc")
                for n in range(N_CHUNKS):
                    ps = mmps.tile([P, NCH], F32, tag="mm")
                    for k in range(K_TILES):
                        nc.tensor.matmul(
                            ps[:],
                            lhsT=UT[k][:, m * P:(m + 1) * P],
                            rhs=UT[k][:, n * NCH:(n + 1) * NCH],
                            start=(k == 0), stop=(k == K_TILES - 1),
                        )
                    if n == m // 4:
                        # own diagonal: zero it (exp then contributes 1)
                        p4 = m % 4
                        nc.vector.tensor_mul(
                            mscr[:], ps[:], maskt[:, p4 * NCH:(p4 + 1) * NCH])
                        src = mscr
                    else:
                        src = ps
                    nc.scalar.activation(
                        escr[:], src[:], AF.Exp, accum_out=racc[:, n:n + 1])
                nc.vector.reduce_sum(rstile[:, m:m + 1], racc[:],
                                     mybir.AxisListType.X)

            nc.sync.dma_start(rsout[:], rstile[:])

    nc.compile()
    _CACHE["nc"] = nc
    return nc


def _host_inputs(embeddings: np.ndarray):
    emb = np.ascontiguousarray(np.asarray(embeddings, dtype=np.float32))
    assert emb.shape == (TWO_N, D)
    # fp32 normalize (matches reference: norm clamped at eps, inactive here)
    norms = np.sqrt((emb.astype(np.float64) ** 2).sum(axis=1, keepdims=True))
    unit = (emb / norms).astype(np.float32)
    up = (unit * np.float32(1.0 / np.sqrt(TEMP))).astype(np.float32)
    d = (up.astype(np.float64) ** 2).sum(axis=1)      # ~= 20, (8192,)
    upT_bf16 = np.ascontiguousarray(up.T.astype(ml_dtypes.bfloat16))

    masks = np.ones((P, 4 * NCH), dtype=np.float32)
    r = np.arange(P)
    for p4 in range(4):
        masks[r, p4 * NCH + p4 * P + r] = 0.0

    in_maps = []
    d_rolled = []
    for c in range(N_CORES):
        s = c * BLK
        utc = np.concatenate([upT_bf16[:, s:], upT_bf16[:, :s]], axis=1)
        m = {f"ut{k}": np.ascontiguousarray(utc[k * P:(k + 1) * P])
             for k in range(K_TILES)}
        m["masks"] = masks
        in_maps.append(m)
        d_rolled.append(np.concatenate([d[s:], d[:s]])[:BLK])
    return in_maps, d_rolled


def run(embeddings: np.ndarray, trace: bool = False):
    nc = _build()
    in_maps, d_rolled = _host_inputs(embeddings)
    res = run_bass_kernel_spmd(nc, in_maps, list(range(N_CORES)), trace=trace)
    total = 0.0
    for c in range(N_CORES):
        rs = np.asarray(res.results[c]["rsout"], dtype=np.float64)  # (128, 8)
        rs_rows = rs.T.reshape(-1)        # row-major over the core's block
        d = d_rolled[c]                   # (1024,)
        se = 2.0 * (rs_rows - 1.0) + np.exp(d)
        total += np.log(se[0::2]).sum() - np.log(d[1::2]).sum()
    loss = np.array(total / (TWO_N // 2), dtype=np.float32)
    return loss, res


def kernel(embeddings: np.ndarray) -> np.ndarray:
    loss, _ = run(embeddings, trace=False)
    return loss
